# revision 5
# baseline (speedup 1.0000x reference)
"""
MultiHeadCrossAttention Trainium2 kernel v2 (Bass/Tile), data-parallel over
batch on 8 NeuronCores.

Key change vs v1: the scores d-reduction is moved off the DVE onto the PE.

  - Q and K projections run with the WEIGHT as the stationary operand, so the
    PSUM output is feature-on-partition: chunk h of Q is Q_h^T [d=128, b=512]
    (d-major), i.e. the transpose needed for PE-side score reduction is free.
  - scores products P_hg[d, b] = Q_h^T * K_g^T are one DVE tensor_tensor per
    head (broadcast over g), still at the fp16 2x rate.
  - The d-reduction for pair j=(h,g) is a matmul with a one-hot-column ones
    stationary: lhsT[:, j] = 1, so out[j, :] += sum_d P[d, :] while all other
    rows get += 0.  64 accumulating matmuls build the full [64, 512] score
    block in a single PSUM bank, replacing the DVE reduce tree entirely.
  - softmax: ACT exp PSUM->SBUF, denominator via one [64x8] group-matrix
    matmul, 1/den via ACT ln/exp, then 4 PE transposes bring e and 1/den back
    to batch-major for the attend stage.
  - V projection and the attend stage stay batch-major as in v1; the last two
    attend tree levels and the residual add run on GpSimd to unload the DVE.
"""

import functools
import sys

import numpy as np

sys.path.insert(0, "/opt/trn_rl_repo")

import concourse.bass as bass  # noqa: E402
import concourse.tile as tile  # noqa: E402
from concourse import bacc, bass_utils, mybir  # noqa: E402


def _patch_act_tables():
    """Force every activation we use (Exp/Ln/Square/Copy/Identity) to resolve
    to the one table set that holds them all (natural_log_exp_and_others), so
    bacc emits a single ACT table load instead of thrashing (1.28us/swap)."""
    import concourse.hw_specs as hw_specs

    orig = hw_specs.get_activation_tables
    if getattr(orig, "_mhca_patched", False):
        return

    A = mybir.ActivationFunctionType
    KEEP = "natural_log_exp_and_others"

    @functools.cache
    def patched(arch):
        tabs = {k: set(v) for k, v in orig(arch).items()}
        for k, s in tabs.items():
            if k != KEEP:
                for f in (A.Exp, A.Ln, A.Square, A.Copy, A.Identity):
                    s.discard(f)
        return tabs

    patched._mhca_patched = True
    hw_specs.get_activation_tables = patched
    import concourse.bass_interp as _bi

    _bi.get_activation_tables = patched
    bacc.get_activation_tables = patched


_patch_act_tables()

# Problem constants (hardcoded per contest contract)
B = 16384
N_CORES = 8
B_LOC = B // N_CORES  # 2048
TEXT_DIM = 1024
IMAGE_DIM = 2048
H = 8
HD = 128
NTC = TEXT_DIM // 128  # 8 text d-chunks
NIC = IMAGE_DIM // 128  # 16 image d-chunks

BT = 128  # batch tile (partition dim)
SW = 512  # slab width (PSUM bank / moving-N limit)
TPS = SW // BT  # tiles per slab = 4

F16 = mybir.dt.float16
F32 = mybir.dt.float32
F8 = mybir.dt.float8e4

INV_SQRT_HD = 1.0 / np.sqrt(128.0)
W_SCALE = 4096.0  # fp8 weight pre-scale (keeps w out of e4m3 subnormals)
DR = mybir.MatmulPerfMode.DoubleRow

# V feature permutation: f' = d*8 + g for original f = g*128 + d, i.e. V is
# stored with the 8 head values of each hidden position adjacent, so the
# attend product / g-reduction reads contiguous 8-element runs.
_d, _g = np.meshgrid(np.arange(128), np.arange(8), indexing="ij")
V_PERM = (_g * 128 + _d).reshape(-1)  # V_PERM[f'] = original f


def build_bass(b_loc: int = B_LOC, apply_affine: bool = False) -> bass.Bass:
    ns = b_loc // SW  # slabs per core

    nc = bacc.Bacc(trn_type="TRN2", debug=False, name="mhca_v2", num_swdge_queues=4)

    # ---- DRAM I/O ----
    text_t = nc.dram_tensor("text_t", [TEXT_DIM, b_loc], F8, kind="ExternalInput")
    image_t = nc.dram_tensor("image_t", [IMAGE_DIM, b_loc], F8, kind="ExternalInput")
    text = nc.dram_tensor("text", [b_loc, TEXT_DIM], F16, kind="ExternalInput")
    wq_t = nc.dram_tensor("wq_t", [TEXT_DIM, TEXT_DIM], F8, kind="ExternalInput")
    wk_t = nc.dram_tensor("wk_t", [IMAGE_DIM, TEXT_DIM], F8, kind="ExternalInput")
    wv_t = nc.dram_tensor("wv_t", [IMAGE_DIM, TEXT_DIM], F8, kind="ExternalInput")
    bqd = nc.dram_tensor("bqd", [128, NTC], F32, kind="ExternalInput")
    bkd = nc.dram_tensor("bkd", [128, H], F32, kind="ExternalInput")
    bv = nc.dram_tensor("bv", [1, TEXT_DIM], F16, kind="ExternalInput")
    oh = nc.dram_tensor("oh", [128, 128], F16, kind="ExternalInput")
    gmat = nc.dram_tensor("gmat", [64, H], F16, kind="ExternalInput")
    ident = nc.dram_tensor("ident", [128, 128], F16, kind="ExternalInput")
    gamma = nc.dram_tensor("gamma", [1, TEXT_DIM], F32, kind="ExternalInput")
    beta = nc.dram_tensor("beta", [1, TEXT_DIM], F32, kind="ExternalInput")
    y = nc.dram_tensor("y", [b_loc, TEXT_DIM], F32, kind="ExternalOutput")

    with tile.TileContext(nc) as tc:
        _body(nc, tc, locals(), ns=ns, apply_affine=apply_affine)
    nc.compile()
    return nc


def _ap(t: bass.AP, dims, off: int = 0) -> bass.AP:
    """Raw AP on an SBUF tile: keep its partition dim, custom free dims."""
    return bass.AP(
        tensor=t.tensor,
        offset=t.offset + off,
        ap=[list(t.ap[0])] + [list(d) for d in dims],
    )


def _body(nc: bass.Bass, tc: tile.TileContext, io: dict, *, ns: int, apply_affine: bool):
    text_t, image_t, text = io["text_t"], io["image_t"], io["text"]
    wq_t, wk_t, wv_t = io["wq_t"], io["wk_t"], io["wv_t"]
    bqd, bkd, bv = io["bqd"], io["bkd"], io["bv"]
    oh, gmat, ident = io["oh"], io["gmat"], io["ident"]
    gamma, beta, y = io["gamma"], io["beta"], io["y"]

    import contextlib

    ctx = contextlib.ExitStack()
    with ctx:
        consts = ctx.enter_context(tc.tile_pool(name="consts", bufs=1))
        slabs = ctx.enter_context(tc.tile_pool(name="slabs", bufs=2))
        kqp = ctx.enter_context(tc.tile_pool(name="kqp", bufs=2))
        qtp = ctx.enter_context(tc.tile_pool(name="qtp", bufs=3))
        sprods = ctx.enter_context(tc.tile_pool(name="sprods", bufs=4))
        estp = ctx.enter_context(tc.tile_pool(name="estp", bufs=2))
        eap = ctx.enter_context(tc.tile_pool(name="eap", bufs=8))
        qkv = ctx.enter_context(tc.tile_pool(name="qkv", bufs=4))
        work = ctx.enter_context(tc.tile_pool(name="work", bufs=2))
        prods = ctx.enter_context(tc.tile_pool(name="prods", bufs=2))  # [128, 4096] halves
        scr2p = ctx.enter_context(tc.tile_pool(name="scr2p", bufs=2))
        outs = ctx.enter_context(tc.tile_pool(name="outs", bufs=2))
        small = ctx.enter_context(tc.tile_pool(name="small", bufs=3))
        psum_kq = ctx.enter_context(tc.tile_pool(name="psum_kq", bufs=2, space="PSUM"))
        psum_sc = ctx.enter_context(tc.tile_pool(name="psum_sc", bufs=2, space="PSUM"))
        psum_d = ctx.enter_context(tc.tile_pool(name="psum_d", bufs=1, space="PSUM"))
        psum_v = ctx.enter_context(tc.tile_pool(name="psum_v", bufs=2, space="PSUM"))

        # ---- constants / weights (fp8, host-quantized x W_SCALE) ----
        w16_q = consts.tile([128, NTC, TEXT_DIM], F8)
        w16_k = consts.tile([128, NIC, TEXT_DIM], F8)
        w16_v = consts.tile([128, NIC, TEXT_DIM], F8)
        wq_r = wq_t[:].rearrange("(c p) f -> p c f", p=128)
        wk_r = wk_t[:].rearrange("(c p) f -> p c f", p=128)
        wv_r = wv_t[:].rearrange("(c p) f -> p c f", p=128)
        def load_w(w16, wr, nch, by_col=False):
            if by_col:
                for g in range(H):
                    nc.sync.dma_start(
                        out=w16[:, :, g * 128 : (g + 1) * 128],
                        in_=wr[:, :, g * 128 : (g + 1) * 128],
                    )
            else:
                for c0 in range(0, nch, 4):
                    nc.sync.dma_start(out=w16[:, c0 : c0 + 4, :], in_=wr[:, c0 : c0 + 4, :])

        # small constants
        oh16 = consts.tile([128, 128], F16)
        nc.sync.dma_start(out=oh16, in_=oh[:])
        g16 = consts.tile([64, H], F16)
        nc.sync.dma_start(out=g16, in_=gmat[:])
        id16 = consts.tile([128, 128], F16)
        nc.sync.dma_start(out=id16, in_=ident[:])
        bq_sb = consts.tile([128, NTC], F32)
        nc.sync.dma_start(out=bq_sb, in_=bqd[:])
        bk_sb = consts.tile([128, H], F32)
        nc.sync.dma_start(out=bk_sb, in_=bkd[:])
        # bv replicated across partitions (pre-scaled x W_SCALE on host), used
        # to pre-fill the V PSUM banks via ACT.
        b16_rep = consts.tile([128, TEXT_DIM], F16)
        nc.sync.dma_start(
            out=b16_rep,
            in_=bass.AP(tensor=bv[:].tensor, offset=0, ap=[[0, 128], [1, TEXT_DIM]]),
        )
        eps_sb = consts.tile([128, 1], F32)
        nc.vector.memset(eps_sb, 1e-5)
        if apply_affine:
            gamma_rep = consts.tile([128, TEXT_DIM], F16)
            beta_rep = consts.tile([128, TEXT_DIM], F16)
            nc.gpsimd.dma_start(
                out=gamma_rep,
                in_=bass.AP(tensor=gamma[:].tensor, offset=0, ap=[[0, 128], [1, TEXT_DIM]]),
            )
            nc.gpsimd.dma_start(
                out=beta_rep,
                in_=bass.AP(tensor=beta[:].tensor, offset=0, ap=[[0, 128], [1, TEXT_DIM]]),
            )

        unscale = 1.0 / W_SCALE

        # ------------- slab phases -------------
        def proj_start(s):
            """Load X^T slabs and allocate the K/Q destination tiles."""
            b0 = s * SW
            xt_text = slabs.tile([128, NTC, SW], F8, tag="xt_text")
            xt_img = slabs.tile([128, NIC, SW], F8, tag="xt_img")
            nc.sync.dma_start(
                out=xt_text,
                in_=text_t[:, b0 : b0 + SW].rearrange("(c p) b -> p c b", p=128),
            )
            nc.sync.dma_start(
                out=xt_img,
                in_=image_t[:, b0 : b0 + SW].rearrange("(c p) b -> p c b", p=128),
            )
            k16T = kqp.tile([128, H, SW], F16, tag="k16T")
            return dict(xt_text=xt_text, xt_img=xt_img, k16T=k16T, q16T=[None] * H)

        def proj_chunks(ph, jt):
            """Emit K chunks 2jt..2jt+1 of this slab."""
            xt_img, k16T = ph["xt_img"], ph["k16T"]
            for g in (2 * jt, 2 * jt + 1):
                pk = psum_kq.tile([128, SW], F32, tag="kq")
                for c in range(0, NIC, 2):
                    nc.tensor.matmul(
                        pk,
                        lhsT=w16_k[:, c : c + 2, g * 128 : (g + 1) * 128],
                        rhs=xt_img[:, c : c + 2, :],
                        start=(c == 0),
                        stop=(c == NIC - 2),
                        perf_mode=DR,
                    )
                nc.scalar.activation(
                    out=k16T[:, g, :],
                    in_=pk,
                    func=mybir.ActivationFunctionType.Identity,
                    scale=unscale,
                    bias=bk_sb[:, g : g + 1],
                )

        def q_proj(ph, h):
            pq = psum_kq.tile([128, SW], F32, tag="kq")
            for c in range(0, NTC, 2):
                nc.tensor.matmul(
                    pq,
                    lhsT=w16_q[:, c : c + 2, h * 128 : (h + 1) * 128],
                    rhs=ph["xt_text"][:, c : c + 2, :],
                    start=(c == 0),
                    stop=(c == NTC - 2),
                    perf_mode=DR,
                )
            q16T = qtp.tile([128, SW], F16, tag="q16T")
            nc.scalar.activation(
                out=q16T,
                in_=pq,
                func=mybir.ActivationFunctionType.Identity,
                scale=unscale,
                bias=bq_sb[:, h : h + 1],
            )
            ph["q16T"][h] = q16T

        def scores_phase(ph):
            """products + one-hot reduce matmuls + softmax + transposes.
            Q-projections run two heads ahead of the sprods so the DVE never
            waits on the ACT drain of the head it is about to multiply."""
            k16T = ph["k16T"]
            sc = psum_sc.tile([64, SW], F32, tag="scores")
            hprods = []

            def ones_mms(hh):
                prod = hprods[hh]
                for g in range(H):
                    j = hh * H + g
                    nc.tensor.matmul(
                        sc,
                        lhsT=oh16[:, 63 - j : 127 - j],
                        rhs=prod[:, g, :],
                        start=(j == 0),
                        stop=(j == 63),
                        skip_group_check=True,
                    )

            q_proj(ph, 0)
            q_proj(ph, 1)
            for h in range(H):
                if h + 2 < H:
                    q_proj(ph, h + 2)
                prod = sprods.tile([128, H, SW], F16, tag="sprod")
                nc.vector.tensor_tensor(
                    out=_ap(prod, [[SW, H], [1, SW]]),
                    in0=_ap(ph["q16T"][h], [[0, H], [1, SW]]),
                    in1=_ap(k16T, [[SW, H], [1, SW]]),
                    op=mybir.AluOpType.mult,
                )
                hprods.append(prod)
                if h > 0:
                    ones_mms(h - 1)
            ones_mms(H - 1)

            est = estp.tile([72, SW], F16, tag="est")
            nc.scalar.activation(
                out=est[0:64, :],
                in_=sc,
                func=mybir.ActivationFunctionType.Exp,
                scale=float(INV_SQRT_HD),
            )
            dn = psum_d.tile([8, SW], F32, tag="den")
            nc.tensor.matmul(dn, lhsT=g16[:], rhs=est[0:64, :], start=True, stop=True)
            ld = estp.tile([8, SW], F32, tag="ld")
            nc.scalar.activation(out=ld, in_=dn, func=mybir.ActivationFunctionType.Ln)
            nc.scalar.activation(
                out=est[64:72, :], in_=ld, func=mybir.ActivationFunctionType.Exp, scale=-1.0
            )

            eas = []
            tp = psum_d.tile([128, 4 * 72], F16, tag="tp")
            for blk in range(TPS):
                nc.tensor.transpose(
                    out=tp[:, blk * 72 : (blk + 1) * 72],
                    in_=est[:, blk * 128 : (blk + 1) * 128],
                    identity=id16[0:72, 0:72],
                )
                ea = eap.tile([128, 72], F16, tag="ea")
                nc.scalar.copy(out=ea, in_=tp[:, blk * 72 : (blk + 1) * 72])
                eas.append(ea)
            ph["eas"] = eas
            return ph

        # ------------- tile phase: V projection + attend + LN -------------
        def stage_a(s, jt, sl):
            """V projection + text load for tile (s, jt)."""
            it = s * TPS + jt
            row0 = it * BT
            xt_img = sl["xt_img"]
            tsl = slice(jt * BT, (jt + 1) * BT)

            text_sb = work.tile([128, TEXT_DIM], F16, tag="text_sb")
            nc.sync.dma_start(out=text_sb, in_=text[row0 : row0 + BT, :])

            # V projection: batch-major, f-halves sequential (1 live PSUM bank)
            vt16 = qkv.tile([128, TEXT_DIM], F16, tag="vt16")
            for f in range(2):
                pv = psum_v.tile([128, 512], F32, tag="v")
                nc.scalar.copy(out=pv, in_=b16_rep[:, f * 512 : (f + 1) * 512])
                for c in range(0, NIC, 2):
                    nc.tensor.matmul(
                        pv,
                        lhsT=xt_img[:, c : c + 2, tsl],
                        rhs=w16_v[:, c : c + 2, f * 512 : (f + 1) * 512],
                        start=False,
                        stop=(c == NIC - 2),
                        perf_mode=DR,
                    )
                nc.scalar.activation(
                    out=vt16[:, f * 512 : (f + 1) * 512],
                    in_=pv,
                    func=mybir.ActivationFunctionType.Identity,
                    scale=unscale,
                )
            return dict(it=it, ea=sl["eas"][jt], text_sb=text_sb, vt16=vt16)

        def stage_b(t):
            """softmax weights + attend + residual."""
            ea, vt16 = t["ea"], t["vt16"]
            a16 = small.tile([128, H * H], F16, tag="a16")
            nc.vector.tensor_tensor(
                out=a16[:].rearrange("p (h g) -> p h g", h=H),
                in0=_ap(ea, [[1, H], [0, H]], off=64),
                in1=_ap(ea, [[8, H], [1, H]]),
                op=mybir.AluOpType.mult,
            )
            prod = prods.tile([128, H * HD * H], F16, tag="prod")
            nc.vector.tensor_tensor(
                out=prod[:].rearrange("p (h d g) -> p h d g", h=H, d=HD),
                in0=_ap(a16, [[8, 8], [0, 128], [1, 8]]),
                in1=_ap(vt16, [[0, 8], [8, 128], [1, 8]]),
                op=mybir.AluOpType.mult,
            )
            scr2 = scr2p.tile([128, H * HD * 4], F16, tag="scr2")
            nc.vector.tensor_tensor(
                out=_ap(scr2, [[4, H * HD], [1, 4]]),
                in0=_ap(prod, [[8, H * HD], [1, 4]]),
                in1=bass.AP(
                    tensor=prod.tensor,
                    offset=prod.offset + 4,
                    ap=[list(prod.ap[0]), [8, H * HD], [1, 4]],
                ),
                op=mybir.AluOpType.add,
            )
            nc.vector.tensor_tensor(
                out=_ap(prod, [[2, H * HD], [1, 2]]),
                in0=_ap(scr2, [[4, H * HD], [1, 2]]),
                in1=bass.AP(
                    tensor=scr2.tensor,
                    offset=scr2.offset + 2,
                    ap=[list(scr2.ap[0]), [4, H * HD], [1, 2]],
                ),
                op=mybir.AluOpType.add,
            )
            att16 = work.tile([128, TEXT_DIM], F16, tag="att16")
            nc.vector.tensor_tensor(
                out=att16,
                in0=_ap(prod, [[2, H * HD]]),
                in1=bass.AP(
                    tensor=prod.tensor,
                    offset=prod.offset + 1,
                    ap=[list(prod.ap[0]), [2, H * HD]],
                ),
                op=mybir.AluOpType.add,
            )
            x = work.tile([128, TEXT_DIM], F16, tag="x")
            nc.vector.tensor_tensor(
                out=x, in0=t["text_sb"], in1=att16, op=mybir.AluOpType.add
            )
            t["x"] = x

        def stage_c(t):
            """LayerNorm + store."""
            x = t["x"]
            row0 = t["it"] * BT
            stats = small.tile([128, 2, 6], F32, tag="stats")
            nc.vector.bn_stats(out=stats[:, 0, :], in_=x[:, 0:512])
            nc.vector.bn_stats(out=stats[:, 1, :], in_=x[:, 512:1024])
            mv = small.tile([128, 2], F32, tag="mv")
            nc.vector.bn_aggr(out=mv, in_=stats)
            negm = small.tile([128, 1], F32, tag="negm")
            nc.vector.tensor_scalar(
                out=negm,
                in0=mv[:, 0:1],
                scalar1=-1.0,
                scalar2=0.0,
                op0=mybir.AluOpType.mult,
                op1=mybir.AluOpType.add,
            )
            lnv = small.tile([128, 1], F32, tag="lnv")
            nc.scalar.activation(
                out=lnv,
                in_=mv[:, 1:2],
                func=mybir.ActivationFunctionType.Ln,
                bias=eps_sb,
                scale=1.0,
            )
            rs = small.tile([128, 1], F32, tag="rs")
            nc.scalar.activation(
                out=rs, in_=lnv, func=mybir.ActivationFunctionType.Exp, scale=-0.5
            )
            nmr = small.tile([128, 1], F32, tag="nmr")
            nc.scalar.activation(
                out=nmr,
                in_=negm,
                func=mybir.ActivationFunctionType.Identity,
                scale=rs,
            )
            if apply_affine:
                xn16 = work.tile([128, TEXT_DIM], F16, tag="xn16")
                nc.scalar.activation(
                    out=xn16,
                    in_=x,
                    func=mybir.ActivationFunctionType.Identity,
                    scale=rs,
                    bias=nmr,
                )
                nc.gpsimd.tensor_tensor(
                    out=xn16, in0=xn16, in1=gamma_rep, op=mybir.AluOpType.mult
                )
                y16 = outs.tile([128, TEXT_DIM], F16, tag="y16")
                nc.gpsimd.tensor_tensor(
                    out=y16, in0=xn16, in1=beta_rep, op=mybir.AluOpType.add
                )
                nc.gpsimd.dma_start(out=y[row0 : row0 + BT, :], in_=y16)
            else:
                y16 = outs.tile([128, TEXT_DIM], F16, tag="y16")
                nc.scalar.activation(
                    out=y16,
                    in_=x,
                    func=mybir.ActivationFunctionType.Identity,
                    scale=rs,
                    bias=nmr,
                )
                nc.gpsimd.dma_start(out=y[row0 : row0 + BT, :], in_=y16)

        # ---- main loop (v2.1 order; ramp blocks flip tiles-first) ----
        pend = []
        prev = None

        def tile_iter(sp, jt, sl):
            t = stage_a(sp, jt, sl)
            if pend:
                stage_b(pend[-1])
            if len(pend) >= 2:
                stage_c(pend[-2])
            pend.append(t)

        for s in range(ns):
            ph = proj_start(s)
            if s == 0:
                # slab-0 X^T lands first, then the weights it needs; Wv is
                # deferred until after the scores phase (V runs a slab later)
                load_w(w16_k, wk_r, NIC, by_col=True)
                load_w(w16_q, wq_r, NTC, by_col=True)
            for jt in range(TPS):
                proj_chunks(ph, jt)
            # ramp blocks: the DVE has no backlog yet, so emit the previous
            # slab's tiles BEFORE this slab's scores phase to avoid the DVE
            # head-of-line idling on the K/Q projections
            flip = s in (1, 2)
            if flip and prev is not None:
                for jt in range(TPS):
                    tile_iter(s - 1, jt, prev)
            cur = scores_phase(ph)
            if s == 0:
                load_w(w16_v, wv_r, NIC)
            if not flip and prev is not None:
                for jt in range(TPS):
                    tile_iter(s - 1, jt, prev)
            prev = cur
        for jt in range(TPS):
            tile_iter(ns - 1, jt, prev)
        stage_b(pend[-1])
        stage_c(pend[-2])
        stage_c(pend[-1])

    io["y"] = y


@functools.lru_cache(maxsize=2)
def _built(b_loc: int, apply_affine: bool = False):
    return build_bass(b_loc, apply_affine)


def _shard_inputs(inputs: dict, b_loc: int, n_cores: int):
    import ml_dtypes

    f32 = lambda a: np.ascontiguousarray(np.asarray(a), dtype=np.float32)
    f16 = lambda a: np.ascontiguousarray(np.asarray(a), dtype=np.float16)
    text = f32(inputs["text_features"])
    image = f32(inputs["image_features"])
    f8 = lambda a: np.ascontiguousarray(
        np.clip(np.asarray(a, np.float32), -240, 240).astype(ml_dtypes.float8_e4m3fn)
    )
    ws = np.float32(W_SCALE)
    wq_t = f8(np.asarray(inputs["Wq"], np.float32).T * ws)
    wk_t = f8(np.asarray(inputs["Wk"], np.float32).T * ws)
    wv_t = f8(np.asarray(inputs["Wv"], np.float32).T[:, V_PERM] * ws)
    # d-major per-partition biases for the ACT drain of Q^T / K^T chunks
    bqd = f32(np.asarray(inputs["bq"], np.float32).reshape(NTC, 128).T)
    bkd = f32(np.asarray(inputs["bk"], np.float32).reshape(H, 128).T)
    bv = f16((np.asarray(inputs["bv"], np.float32)[V_PERM] * ws).reshape(1, -1))
    gm = f32(inputs["gamma"]).reshape(1, -1)
    bt = f32(inputs["beta"]).reshape(1, -1)

    # one-hot sliding-window matrix: OH[p, c] = 1 iff c == 63
    oh = np.zeros((128, 128), np.float16)
    oh[:, 63] = 1.0
    # group matrix: G[i, m] = 1 iff i // 8 == m
    gmat = np.zeros((64, H), np.float16)
    for i in range(64):
        gmat[i, i // H] = 1.0
    ident = np.eye(128, dtype=np.float16)

    in_maps = []
    for c in range(n_cores):
        sl = slice(c * b_loc, (c + 1) * b_loc)
        in_maps.append(
            {
                "text_t": f8(text[sl].T),
                "image_t": f8(image[sl].T),
                "text": f16(text[sl]),
                "wq_t": wq_t,
                "wk_t": wk_t,
                "wv_t": wv_t,
                "bqd": bqd,
                "bkd": bkd,
                "bv": bv,
                "oh": oh,
                "gmat": gmat,
                "ident": ident,
                "gamma": gm,
                "beta": bt,
            }
        )
    return in_maps


def kernel(**inputs) -> np.ndarray:
    affine = not (
        np.allclose(np.asarray(inputs["gamma"]), 1.0)
        and np.allclose(np.asarray(inputs["beta"]), 0.0)
    )
    nc = _built(B_LOC, affine)
    in_maps = _shard_inputs(inputs, B_LOC, N_CORES)
    res = bass_utils.run_bass_kernel_spmd(nc, in_maps, core_ids=list(range(N_CORES)))
    return np.concatenate([r["y"] for r in res.results], axis=0)
